# revision 15
# baseline (speedup 1.0000x reference)
"""Trainium2 Bass kernel for causal ReLU attention (no softmax).

  qkv = x @ W.T + b;  per head: s = (q k^T) * 1/sqrt(64)
  p = relu(causal(s));  y = p @ v

Sharding: 8 cores = 2 batches x 4 head-groups (3 heads each). Each core:
  - qk-projection computed transposed (features on partitions) so q/k land
    as qT/kT [64, T] ready to be matmul operands with d on partitions
  - v-projection computed natural [T, 192]
  - scores: K=128 zero-padded contraction (two heads share a 128-partition
    tile; lhsT = [kT_h; 0] makes each head's matmul full-width)
  - block-causal: fully-masked key blocks skipped, diagonal blocks get a
    restricted column range + triangle mask via one fused DVE op
All matmul operands fp16 (fp32 PSUM accumulation). Host does the
shard/transpose/cast prep and the final gather (pure numpy).
"""
import numpy as np

import concourse.bass as bass
import concourse.mybir as mybir
import concourse.tile as tile
from concourse import bacc
from concourse.bass_utils import run_bass_kernel_spmd

F32 = mybir.dt.float32
F16 = mybir.dt.float16

B, T, C = 2, 2048, 768
NH = 12          # total heads
HPC = 3          # heads per core
D = 64
NCORES = 8
CC = 6           # contraction chunks (768 / 128)
TB = 512         # query block
KB = 128         # key block
NTB = T // TB    # 4
NKB = T // KB    # 16


def _build(reps=1, stage=4):
    nc = bacc.Bacc(None, target_bir_lowering=False, debug=False)
    xT = nc.declare_dram_parameter("xT", [C, T], F16, isOutput=False)
    wqk = nc.declare_dram_parameter("wqk", [C, 384], F16, isOutput=False)
    wv = nc.declare_dram_parameter("wv", [C, 192], F16, isOutput=False)
    bias_qk = nc.declare_dram_parameter("bias_qk", [3, 128], F32, isOutput=False)
    scale_qk = nc.declare_dram_parameter("scale_qk", [3, 128], F32, isOutput=False)
    bias_v = nc.declare_dram_parameter("bias_v", [128, 192], F32, isOutput=False)
    yt_out = nc.declare_dram_parameter("yt", [HPC, D, T], F32, isOutput=True)

    with tile.TileContext(nc) as tc:
        with tc.tile_pool(name="const", bufs=1) as const, \
             tc.tile_pool(name="xr", bufs=12) as xr, \
             tc.tile_pool(name="qk", bufs=5) as qkp, \
             tc.tile_pool(name="vt", bufs=16) as vtp, \
             tc.tile_pool(name="pt", bufs=4) as ptp, \
             tc.tile_pool(name="ys", bufs=2) as ysp, \
             tc.tile_pool(name="psmix", bufs=2, space="PSUM") as psmix, \
             tc.tile_pool(name="pssc", bufs=3, space="PSUM") as pssc:

            # ---------------- constants ----------------
            bias_sb = const.tile([128, 3], F32)
            scale_sb = const.tile([128, 3], F32)
            nc.sync.dma_start(out=bias_sb, in_=bias_qk[:, :].rearrange("a p -> p a"))
            nc.sync.dma_start(out=scale_sb, in_=scale_qk[:, :].rearrange("a p -> p a"))
            biasv_sb = const.tile([128, 192], F32)
            nc.sync.dma_start(out=biasv_sb, in_=bias_v[:, :])
            # triangle mask M[kk, qq] = 1 if qq >= kk else 0  (f32: faster DVE read)
            mask_sb = const.tile([128, TB], F32)
            nc.vector.memset(mask_sb, 1.0)
            nc.gpsimd.affine_select(
                out=mask_sb, in_=mask_sb,
                compare_op=mybir.AluOpType.is_ge, fill=0.0, base=0,
                pattern=[[1, TB]], channel_multiplier=-1)
            ones2_sb = const.tile([128, 2 * TB], F32)
            nc.vector.memset(ones2_sb, 1.0)

            # weight chunks
            wqk_sb = [const.tile([128, 384], F16, tag="wqk", bufs=CC, name=f"wqk{c}") for c in range(CC)]
            wv_sb = [const.tile([128, 192], F16, tag="wv", bufs=CC, name=f"wv{c}") for c in range(CC)]
            for c in range(CC):
                nc.sync.dma_start(out=wqk_sb[c], in_=wqk[c * 128:(c + 1) * 128, :])
                nc.sync.dma_start(out=wv_sb[c], in_=wv[c * 128:(c + 1) * 128, :])

            # persistent attention operand tiles
            qq01 = const.tile([128, T], F16)   # [qT_h0; qT_h1]
            kzA = const.tile([128, T], F16)    # [kT_h0; 0]
            kzB = const.tile([128, T], F16)    # [0; kT_h1]
            qq2 = const.tile([128, T], F16)    # [qT_h2(via dma); qT_h2(act)]
            kz2 = const.tile([128, T], F16)    # [kT_h2; 0]
            nc.vector.memset(kzA[64:128, :], 0.0)
            nc.vector.memset(kzB[0:64, :], 0.0)
            nc.vector.memset(kz2[64:128, :], 0.0)

            def body():
                # stage: 1=dma only, 2=+proj, 3=+scores/relu, 4=full
                # ---------------- load xT (fp16, pre-cast on host) ---------
                xt = [xr.tile([128, T], F16, tag="xt", name=f"xt{c}") for c in range(CC)]
                for c in range(CC):
                    nc.sync.dma_start(out=xt[c], in_=xT[c * 128:(c + 1) * 128, :])

                if stage < 2:
                    return
                # ---------------- qk projection (transposed) ---------------
                # f-tiles: 0 = [q0; q1], 1 = [k0; k1], 2 = [k2; q2]
                Copy = mybir.ActivationFunctionType.Identity
                for ft in range(3):
                    for tb in range(NTB):
                        ps = psmix.tile([128, TB], F32, tag="m")
                        for c in range(CC):
                            nc.tensor.matmul(
                                ps, wqk_sb[c][:, ft * 128:(ft + 1) * 128],
                                xt[c][:, tb * TB:(tb + 1) * TB],
                                start=(c == 0), stop=(c == CC - 1))
                        ts = slice(tb * TB, (tb + 1) * TB)
                        if ft == 0:
                            nc.scalar.activation(qq01[:, ts], ps, Copy,
                                                 bias=bias_sb[:, 0:1],
                                                 scale=scale_sb[:, 0:1])
                        elif ft == 1:
                            nc.scalar.activation(kzA[0:64, ts], ps[0:64, :], Copy,
                                                 bias=bias_sb[0:64, 1:2],
                                                 scale=scale_sb[0:64, 1:2])
                            nc.scalar.activation(kzB[64:128, ts], ps[64:128, :], Copy,
                                                 bias=bias_sb[64:128, 1:2],
                                                 scale=scale_sb[64:128, 1:2])
                        else:
                            nc.scalar.activation(kz2[0:64, ts], ps[0:64, :], Copy,
                                                 bias=bias_sb[0:64, 2:3],
                                                 scale=scale_sb[0:64, 2:3])
                            nc.scalar.activation(qq2[64:128, ts], ps[64:128, :], Copy,
                                                 bias=bias_sb[64:128, 2:3],
                                                 scale=scale_sb[64:128, 2:3])
                # shift qT_h2 to partitions 0-63 (SBUF->SBUF DMA)
                nc.sync.dma_start(out=qq2[0:64, :], in_=qq2[64:128, :])

                # ---------------- v projection (natural layout) ------------
                v_sb = []
                for tt in range(NKB):
                    ps = psmix.tile([128, 192], F32, tag="m")
                    for c in range(CC):
                        nc.tensor.matmul(
                            ps, xt[c][:, tt * 128:(tt + 1) * 128], wv_sb[c],
                            start=(c == 0), stop=(c == CC - 1))
                    vt = vtp.tile([128, 192], F16, tag="v")
                    nc.vector.tensor_add(vt, ps, biasv_sb)
                    v_sb.append(vt)

                if stage < 3:
                    return
                # ---------------- attention ----------------
                heads = [(kzA, qq01), (kzB, qq01), (kz2, qq2)]
                relu_flip = [0]
                ys_head = [ysp.tile([64, T], F32, tag="ys", bufs=3, name=f"ysh{h}")
                           for h in range(3)]
                done_qb = [0, 0, 0]
                order = [(0, 0), (1, 0), (0, 1), (1, 1), (2, 0), (0, 2),
                         (1, 2), (2, 1), (0, 3), (1, 3), (2, 2), (2, 3)]
                for hl, qb in order:
                    kz, qq = heads[hl]
                    if True:
                        ytp = psmix.tile([64, TB], F32, tag="m")
                        nkb = 4 * qb + 4
                        nfull = 4 * qb
                        # full blocks in pairs -> one wide relu per pair
                        for kp in range(nfull // 2):
                            sp2 = pssc.tile([128, 2 * TB], F32, tag="s")
                            for half in range(2):
                                kb = 2 * kp + half
                                nc.tensor.matmul(
                                    sp2[:, half * TB:(half + 1) * TB],
                                    kz[:, kb * KB:(kb + 1) * KB],
                                    qq[:, qb * TB:(qb + 1) * TB],
                                    start=True, stop=True)
                            pt2 = ptp.tile([128, 2 * TB], F16, tag="p2")
                            relu_flip[0] += 1
                            if relu_flip[0] % 5 < 2:
                                nc.vector.scalar_tensor_tensor(
                                    out=pt2, in0=sp2, scalar=0.0, in1=ones2_sb,
                                    op0=mybir.AluOpType.max,
                                    op1=mybir.AluOpType.mult)
                            else:
                                nc.scalar.activation(
                                    pt2, sp2, mybir.ActivationFunctionType.Relu)
                            for half in range(2):
                                kb = 2 * kp + half
                                if stage >= 4:
                                    nc.tensor.matmul(
                                        ytp,
                                        v_sb[kb][:, hl * 64:(hl + 1) * 64],
                                        pt2[:, half * TB:(half + 1) * TB],
                                        start=(kb == 0), stop=False)
                        # diagonal blocks: restricted range + triangle mask
                        # two diagonal blocks share one wide psum tile
                        for dp in range(2):
                            sp2 = pssc.tile([128, 2 * TB], F32, tag="s")
                            pt2 = ptp.tile([128, 2 * TB], F16, tag="p2")
                            for half in range(2):
                                j = 2 * dp + half
                                kb = nfull + j
                                lo = j * KB
                                n = TB - lo
                                off = half * TB
                                nc.tensor.matmul(
                                    sp2[:, off + lo:off + TB],
                                    kz[:, kb * KB:(kb + 1) * KB],
                                    qq[:, qb * TB + lo:(qb + 1) * TB],
                                    start=True, stop=True)
                                nc.vector.scalar_tensor_tensor(
                                    out=pt2[:, off + lo:off + TB],
                                    in0=sp2[:, off + lo:off + TB],
                                    scalar=0.0, in1=mask_sb[:, 0:n],
                                    op0=mybir.AluOpType.max,
                                    op1=mybir.AluOpType.mult)
                                if stage >= 4:
                                    nc.tensor.matmul(
                                        ytp[:, lo:TB],
                                        v_sb[kb][:, hl * 64:(hl + 1) * 64],
                                        pt2[:, off + lo:off + TB],
                                        start=(kb == 0), stop=(kb == nkb - 1))
                        if stage >= 4:
                            ys = ys_head[hl]
                            nc.scalar.activation(
                                ys[:, qb * TB:(qb + 1) * TB], ytp,
                                mybir.ActivationFunctionType.Identity)
                            done_qb[hl] += 1
                            if done_qb[hl] == NTB:
                                nc.sync.dma_start(out=yt_out[hl, :, :], in_=ys)

            if reps == 1:
                body()
            elif reps < 0:
                with tc.For_i(0, -reps, 1):
                    body()
            else:
                for _ in range(reps):
                    body()

    nc.finalize()
    return nc


def _prepare_in_maps(x, W_attn, b_attn):
    x = np.asarray(x, dtype=np.float32)
    W = np.asarray(W_attn, dtype=np.float32)
    bb = np.asarray(b_attn, dtype=np.float32)
    SC = np.float32(1.0 / np.sqrt(D))

    xT16 = [np.ascontiguousarray(x[b].T).astype(np.float16) for b in range(B)]

    in_maps = []
    for core in range(NCORES):
        b, g = divmod(core, NCORES // B)
        H = [g * HPC + h for h in range(HPC)]
        q_rows = [W[h * D:(h + 1) * D] for h in H]
        k_rows = [W[C + h * D:C + (h + 1) * D] for h in H]
        v_rows = [W[2 * C + h * D:2 * C + (h + 1) * D] for h in H]
        bq = [bb[h * D:(h + 1) * D] for h in H]
        bk = [bb[C + h * D:C + (h + 1) * D] for h in H]
        bv = [bb[2 * C + h * D:2 * C + (h + 1) * D] for h in H]

        # f-tiles: 0 = [q0; q1], 1 = [k0; k1], 2 = [k2; q2]
        wqk_rows = np.concatenate(
            [q_rows[0], q_rows[1], k_rows[0], k_rows[1], k_rows[2], q_rows[2]], 0)
        wqk16 = np.ascontiguousarray(wqk_rows.T).astype(np.float16)   # [768, 384]
        wv16 = np.ascontiguousarray(
            np.concatenate(v_rows, 0).T).astype(np.float16)           # [768, 192]

        bias_qk = np.stack([
            np.concatenate([bq[0], bq[1]]) * SC,
            np.concatenate([bk[0], bk[1]]),
            np.concatenate([bk[2], bq[2] * SC]),
        ]).astype(np.float32)                                          # [3, 128]
        scale_qk = np.stack([
            np.full(128, SC), np.ones(128),
            np.concatenate([np.ones(64), np.full(64, SC)]),
        ]).astype(np.float32)
        bias_v = np.tile(np.concatenate(bv), (128, 1)).astype(np.float32)

        in_maps.append({
            "xT": xT16[b], "wqk": wqk16, "wv": wv16,
            "bias_qk": bias_qk, "scale_qk": scale_qk, "bias_v": bias_v,
        })
    return in_maps


_NC_CACHE = {}


def _get_nc(reps=1, stage=4):
    key = (reps, stage)
    if key not in _NC_CACHE:
        _NC_CACHE[key] = _build(reps, stage)
    return _NC_CACHE[key]


def kernel(x, W_attn, b_attn):
    nc = _get_nc(1)
    in_maps = _prepare_in_maps(x, W_attn, b_attn)
    res = run_bass_kernel_spmd(nc, in_maps, list(range(NCORES)), trace=False)
    y = np.empty((B, T, C), dtype=np.float32)
    for core in range(NCORES):
        b, g = divmod(core, NCORES // B)
        yt = res.results[core]["yt"]          # [3, 64, 2048]
        for h in range(HPC):
            y[b, :, (g * HPC + h) * D:(g * HPC + h + 1) * D] = yt[h].T
    return y


# revision 16
# speedup vs baseline: 1.1144x; 1.1144x over previous
"""Trainium2 Bass kernel for causal ReLU attention (no softmax).

  qkv = x @ W.T + b;  per head: s = (q k^T) * 1/sqrt(64)
  p = relu(causal(s));  y = p @ v

Sharding: 8 cores = 2 batches x 4 head-groups (3 heads each). Each core:
  - qk-projection computed transposed (features on partitions) so q/k land
    as qT/kT [64, T] ready to be matmul operands with d on partitions
  - v-projection computed natural [T, 192]
  - scores: K=128 zero-padded contraction (two heads share a 128-partition
    tile; lhsT = [kT_h; 0] makes each head's matmul full-width)
  - block-causal: fully-masked key blocks skipped, diagonal blocks get a
    restricted column range + triangle mask via one fused DVE op
All matmul operands fp16 (fp32 PSUM accumulation). Host does the
shard/transpose/cast prep and the final gather (pure numpy).
"""
import numpy as np

import concourse.bass as bass
import concourse.mybir as mybir
import concourse.tile as tile
from concourse import bacc
from concourse.bass_utils import run_bass_kernel_spmd

F32 = mybir.dt.float32
F16 = mybir.dt.float16

B, T, C = 2, 2048, 768
NH = 12          # total heads
HPC = 3          # heads per core
D = 64
NCORES = 8
CC = 6           # contraction chunks (768 / 128)
TB = 512         # query block
KB = 128         # key block
NTB = T // TB    # 4
NKB = T // KB    # 16


def _build(reps=1, stage=4):
    nc = bacc.Bacc(None, target_bir_lowering=False, debug=False)
    xT = nc.declare_dram_parameter("xT", [C, T], F16, isOutput=False)
    wqk = nc.declare_dram_parameter("wqk", [C, 384], F16, isOutput=False)
    wv = nc.declare_dram_parameter("wv", [C, 192], F16, isOutput=False)
    bias_qk = nc.declare_dram_parameter("bias_qk", [3, 128], F32, isOutput=False)
    scale_qk = nc.declare_dram_parameter("scale_qk", [3, 128], F32, isOutput=False)
    bias_v = nc.declare_dram_parameter("bias_v", [128, 192], F32, isOutput=False)
    yt_out = nc.declare_dram_parameter("yt", [HPC, D, T], F32, isOutput=True)

    with tile.TileContext(nc) as tc:
        with tc.tile_pool(name="const", bufs=1) as const, \
             tc.tile_pool(name="xr", bufs=12) as xr, \
             tc.tile_pool(name="qk", bufs=5) as qkp, \
             tc.tile_pool(name="vt", bufs=16) as vtp, \
             tc.tile_pool(name="pt", bufs=4) as ptp, \
             tc.tile_pool(name="ys", bufs=2) as ysp, \
             tc.tile_pool(name="psmix", bufs=2, space="PSUM") as psmix, \
             tc.tile_pool(name="pssc", bufs=3, space="PSUM") as pssc:

            # ---------------- constants ----------------
            bias_sb = const.tile([128, 3], F32)
            scale_sb = const.tile([128, 3], F32)
            nc.sync.dma_start(out=bias_sb, in_=bias_qk[:, :].rearrange("a p -> p a"))
            nc.sync.dma_start(out=scale_sb, in_=scale_qk[:, :].rearrange("a p -> p a"))
            biasv_sb = const.tile([128, 192], F32)
            nc.sync.dma_start(out=biasv_sb, in_=bias_v[:, :])
            # triangle mask M[kk, qq] = 1 if qq >= kk else 0  (f32: faster DVE read)
            mask_sb = const.tile([128, TB], F32)
            nc.vector.memset(mask_sb, 1.0)
            nc.gpsimd.affine_select(
                out=mask_sb, in_=mask_sb,
                compare_op=mybir.AluOpType.is_ge, fill=0.0, base=0,
                pattern=[[1, TB]], channel_multiplier=-1)
            ones2_sb = const.tile([128, 2 * TB], F32)
            nc.vector.memset(ones2_sb, 1.0)

            # weight chunks
            wqk_sb = [const.tile([128, 384], F16, tag="wqk", bufs=CC, name=f"wqk{c}") for c in range(CC)]
            wv_sb = [const.tile([128, 192], F16, tag="wv", bufs=CC, name=f"wv{c}") for c in range(CC)]
            for c in range(CC):
                nc.sync.dma_start(out=wqk_sb[c], in_=wqk[c * 128:(c + 1) * 128, :])
                nc.sync.dma_start(out=wv_sb[c], in_=wv[c * 128:(c + 1) * 128, :])

            # persistent attention operand tiles
            qq01 = const.tile([128, T], F16)   # [qT_h0; qT_h1]
            kzA = const.tile([128, T], F16)    # [kT_h0; 0]
            kzB = const.tile([128, T], F16)    # [0; kT_h1]
            qq2 = const.tile([128, T], F16)    # [qT_h2(via dma); qT_h2(act)]
            kz2 = const.tile([128, T], F16)    # [kT_h2; 0]
            nc.vector.memset(kzA[64:128, :], 0.0)
            nc.vector.memset(kzB[0:64, :], 0.0)
            nc.vector.memset(kz2[64:128, :], 0.0)

            def body():
                # stage: 1=dma only, 2=+proj, 3=+scores/relu, 4=full
                # ---------------- load xT (fp16, pre-cast on host) ---------
                xt = [xr.tile([128, T], F16, tag="xt", name=f"xt{c}") for c in range(CC)]
                for c in range(CC):
                    nc.sync.dma_start(out=xt[c], in_=xT[c * 128:(c + 1) * 128, :])

                if stage < 2:
                    return
                # ---------------- qk projection (transposed) ---------------
                # f-tiles: 0 = [q0; q1], 1 = [k0; k1], 2 = [k2; q2]
                Copy = mybir.ActivationFunctionType.Identity
                for ft in range(3):
                    for tb in range(NTB):
                        ps = psmix.tile([128, TB], F32, tag="m")
                        for c in range(CC):
                            nc.tensor.matmul(
                                ps, wqk_sb[c][:, ft * 128:(ft + 1) * 128],
                                xt[c][:, tb * TB:(tb + 1) * TB],
                                start=(c == 0), stop=(c == CC - 1))
                        ts = slice(tb * TB, (tb + 1) * TB)
                        if ft == 0:
                            nc.scalar.activation(qq01[:, ts], ps, Copy,
                                                 bias=bias_sb[:, 0:1],
                                                 scale=scale_sb[:, 0:1])
                        elif ft == 1:
                            nc.scalar.activation(kzA[0:64, ts], ps[0:64, :], Copy,
                                                 bias=bias_sb[0:64, 1:2],
                                                 scale=scale_sb[0:64, 1:2])
                            nc.scalar.activation(kzB[64:128, ts], ps[64:128, :], Copy,
                                                 bias=bias_sb[64:128, 1:2],
                                                 scale=scale_sb[64:128, 1:2])
                        else:
                            nc.scalar.activation(kz2[0:64, ts], ps[0:64, :], Copy,
                                                 bias=bias_sb[0:64, 2:3],
                                                 scale=scale_sb[0:64, 2:3])
                            nc.scalar.activation(qq2[64:128, ts], ps[64:128, :], Copy,
                                                 bias=bias_sb[64:128, 2:3],
                                                 scale=scale_sb[64:128, 2:3])
                # shift qT_h2 to partitions 0-63 (SBUF->SBUF DMA)
                nc.sync.dma_start(out=qq2[0:64, :], in_=qq2[64:128, :])

                # ---------------- v projection (natural layout) ------------
                v_sb = []
                for tt in range(NKB):
                    ps = psmix.tile([128, 192], F32, tag="m")
                    for c in range(CC):
                        nc.tensor.matmul(
                            ps, xt[c][:, tt * 128:(tt + 1) * 128], wv_sb[c],
                            start=(c == 0), stop=(c == CC - 1))
                    vt = vtp.tile([128, 192], F16, tag="v")
                    nc.vector.tensor_add(vt, ps, biasv_sb)
                    v_sb.append(vt)

                if stage < 3:
                    return
                # ---------------- attention ----------------
                heads = [(kzA, qq01), (kzB, qq01), (kz2, qq2)]
                relu_flip = [0]
                order = [(0, 0), (1, 0), (0, 1), (1, 1), (2, 0), (0, 2),
                         (1, 2), (2, 1), (0, 3), (1, 3), (2, 2), (2, 3)]
                for hl, qb in order:
                    kz, qq = heads[hl]
                    if True:
                        ytp = psmix.tile([64, TB], F32, tag="m")
                        nkb = 4 * qb + 4
                        nfull = 4 * qb
                        # full blocks in pairs -> one wide relu per pair
                        for kp in range(nfull // 2):
                            sp2 = pssc.tile([128, 2 * TB], F32, tag="s")
                            for half in range(2):
                                kb = 2 * kp + half
                                nc.tensor.matmul(
                                    sp2[:, half * TB:(half + 1) * TB],
                                    kz[:, kb * KB:(kb + 1) * KB],
                                    qq[:, qb * TB:(qb + 1) * TB],
                                    start=True, stop=True)
                            pt2 = ptp.tile([128, 2 * TB], F16, tag="p2")
                            relu_flip[0] += 1
                            if relu_flip[0] % 5 < 2:
                                nc.vector.scalar_tensor_tensor(
                                    out=pt2, in0=sp2, scalar=0.0, in1=ones2_sb,
                                    op0=mybir.AluOpType.max,
                                    op1=mybir.AluOpType.mult)
                            else:
                                nc.scalar.activation(
                                    pt2, sp2, mybir.ActivationFunctionType.Relu)
                            for half in range(2):
                                kb = 2 * kp + half
                                if stage >= 4:
                                    nc.tensor.matmul(
                                        ytp,
                                        v_sb[kb][:, hl * 64:(hl + 1) * 64],
                                        pt2[:, half * TB:(half + 1) * TB],
                                        start=(kb == 0), stop=False)
                        # diagonal blocks: restricted range + triangle mask
                        # two diagonal blocks share one wide psum tile
                        for dp in range(2):
                            sp2 = pssc.tile([128, 2 * TB], F32, tag="s")
                            pt2 = ptp.tile([128, 2 * TB], F16, tag="p2")
                            for half in range(2):
                                j = 2 * dp + half
                                kb = nfull + j
                                lo = j * KB
                                n = TB - lo
                                off = half * TB
                                nc.tensor.matmul(
                                    sp2[:, off + lo:off + TB],
                                    kz[:, kb * KB:(kb + 1) * KB],
                                    qq[:, qb * TB + lo:(qb + 1) * TB],
                                    start=True, stop=True)
                                nc.vector.scalar_tensor_tensor(
                                    out=pt2[:, off + lo:off + TB],
                                    in0=sp2[:, off + lo:off + TB],
                                    scalar=0.0, in1=mask_sb[:, 0:n],
                                    op0=mybir.AluOpType.max,
                                    op1=mybir.AluOpType.mult)
                                if stage >= 4:
                                    nc.tensor.matmul(
                                        ytp[:, lo:TB],
                                        v_sb[kb][:, hl * 64:(hl + 1) * 64],
                                        pt2[:, off + lo:off + TB],
                                        start=(kb == 0), stop=(kb == nkb - 1))
                        if stage >= 4:
                            ys = ysp.tile([64, TB], F32, tag="ys")
                            nc.scalar.activation(
                                ys, ytp, mybir.ActivationFunctionType.Identity)
                            nc.sync.dma_start(
                                out=yt_out[hl, :, qb * TB:(qb + 1) * TB], in_=ys)

            if reps == 1:
                body()
            elif reps < 0:
                with tc.For_i(0, -reps, 1):
                    body()
            else:
                for _ in range(reps):
                    body()

    nc.finalize()
    return nc


def _prepare_in_maps(x, W_attn, b_attn):
    x = np.asarray(x, dtype=np.float32)
    W = np.asarray(W_attn, dtype=np.float32)
    bb = np.asarray(b_attn, dtype=np.float32)
    SC = np.float32(1.0 / np.sqrt(D))

    xT16 = [np.ascontiguousarray(x[b].T).astype(np.float16) for b in range(B)]

    in_maps = []
    for core in range(NCORES):
        b, g = divmod(core, NCORES // B)
        H = [g * HPC + h for h in range(HPC)]
        q_rows = [W[h * D:(h + 1) * D] for h in H]
        k_rows = [W[C + h * D:C + (h + 1) * D] for h in H]
        v_rows = [W[2 * C + h * D:2 * C + (h + 1) * D] for h in H]
        bq = [bb[h * D:(h + 1) * D] for h in H]
        bk = [bb[C + h * D:C + (h + 1) * D] for h in H]
        bv = [bb[2 * C + h * D:2 * C + (h + 1) * D] for h in H]

        # f-tiles: 0 = [q0; q1], 1 = [k0; k1], 2 = [k2; q2]
        wqk_rows = np.concatenate(
            [q_rows[0], q_rows[1], k_rows[0], k_rows[1], k_rows[2], q_rows[2]], 0)
        wqk16 = np.ascontiguousarray(wqk_rows.T).astype(np.float16)   # [768, 384]
        wv16 = np.ascontiguousarray(
            np.concatenate(v_rows, 0).T).astype(np.float16)           # [768, 192]

        bias_qk = np.stack([
            np.concatenate([bq[0], bq[1]]) * SC,
            np.concatenate([bk[0], bk[1]]),
            np.concatenate([bk[2], bq[2] * SC]),
        ]).astype(np.float32)                                          # [3, 128]
        scale_qk = np.stack([
            np.full(128, SC), np.ones(128),
            np.concatenate([np.ones(64), np.full(64, SC)]),
        ]).astype(np.float32)
        bias_v = np.tile(np.concatenate(bv), (128, 1)).astype(np.float32)

        in_maps.append({
            "xT": xT16[b], "wqk": wqk16, "wv": wv16,
            "bias_qk": bias_qk, "scale_qk": scale_qk, "bias_v": bias_v,
        })
    return in_maps


_NC_CACHE = {}


def _get_nc(reps=1, stage=4):
    key = (reps, stage)
    if key not in _NC_CACHE:
        _NC_CACHE[key] = _build(reps, stage)
    return _NC_CACHE[key]


def kernel(x, W_attn, b_attn):
    nc = _get_nc(1)
    in_maps = _prepare_in_maps(x, W_attn, b_attn)
    res = run_bass_kernel_spmd(nc, in_maps, list(range(NCORES)), trace=False)
    y = np.empty((B, T, C), dtype=np.float32)
    for core in range(NCORES):
        b, g = divmod(core, NCORES // B)
        yt = res.results[core]["yt"]          # [3, 64, 2048]
        for h in range(HPC):
            y[b, :, (g * HPC + h) * D:(g * HPC + h + 1) * D] = yt[h].T
    return y
